# revision 1
# baseline (speedup 1.0000x reference)
"""FlowNet-style correlation layer (B=4, C=128, H=W=192, k=9, stride=1) on 8 trn2 cores.

Design (per core; cores = 4 batches x 2 H-halves, SPMD):
  - Host pre-blocks x into per-patch-contiguous layout [c, blk, 128] (bf16) and
    pre-transposes the zero-padded y shard to w-major [c, 200, 104] (bf16).
  - Both live resident in SBUF. Per block-row bh, one copy stages a w-major
    context row-band Yrow [c, (w':200, h':16)] (contiguous dst, unit-stride src).
  - For each 8x16 pixel patch (144 blocks): one PE matmul contracting channels:
    lhsT = x-patch [c, 128], rhs = Yrow slice [c, 384] -> psum[128, 384]
    ("banded all-pairs": psum[m, n] = sum_c x[c,pix_m] * y[c,ctx_n]).
  - Evacuate psum -> sbuf bf16 with the 1/C scale (alternating ACT/DVE).
  - DMA the band SBUF->DRAM with a *sheared* destination AP (dest addr subtracts
    each pixel's context offset pos(m) = wl*16 + hl, linear in (hl, wl, n)), so
    each pixel's 81 useful offsets land at uniform positions q = dj*16 + di.
  - Uniform strided re-read DRAM->SBUF [128, 137], compact [dj:9 x16][di:9] ->
    f32 [128, 81] (cast + di/dj transpose), batch 8 blocks, contiguous DMA to
    out[m, blk, k].
  - Host reassembles [B, 81, 192, 192] from per-core [128, 144, 81].
"""

import numpy as np

B, C, H, W = 4, 128, 192, 192
K = 9                      # kernel_size
PAD = 4                    # displacement radius
NCORES = 8
HSH = H // 2               # 96 rows per core
YH, YW = HSH + 2 * PAD, W + 2 * PAD       # 104, 200
PH, PW = 8, 16             # patch shape (128 pixels)
CH, CW = PH + 2 * PAD, PW + 2 * PAD       # context 16 x 24
NCTX = CH * CW             # 384 band columns
NBH, NBW = HSH // PH, W // PW             # 12 x 12 = 144 blocks
NBLK = NBH * NBW
K2 = K * K                 # 81
QW = (K - 1) * CH + K      # 137: useful offsets q = dj*16 + di in [0, 137)
SHEAR_MAX = CH * (PW - 1) + (PH - 1)      # 247 = max pos(m)
SPITCH = NCTX + SHEAR_MAX + 1             # 632 sheared row pitch
FLUSH = 8                  # blocks per output flush

_nc_cache = None


def _strided_view(dram_t, offset, dims):
    """Arbitrary strided AP over a flat DRAM tensor.

    dims: [(step, count), ...] outer-to-inner, steps in elements."""
    total = 1
    for _, c in dims:
        total *= c
    v = dram_t[:][offset:offset + total]
    if len(dims) > 1:
        names = "abcdefg"[: len(dims)]
        kw = {n: c for n, (_, c) in zip(names[1:], dims[1:])}
        v = v.rearrange(f"({' '.join(names)}) -> {' '.join(names)}", **kw)
    ap = v.ap
    for i, (s, c) in enumerate(dims):
        ap[i] = [s, c]
    v.ap = ap
    return v


def _build_nc():
    import concourse.bacc as bacc
    import concourse.mybir as mybir
    import concourse.tile as tile

    bf16 = mybir.dt.bfloat16
    f32 = mybir.dt.float32

    nc = bacc.Bacc("TRN2", target_bir_lowering=False, debug=False)
    x_d = nc.dram_tensor("x", [C, NBLK * 128], bf16, kind="ExternalInput")
    y_d = nc.dram_tensor("y", [C, YW * YH], bf16, kind="ExternalInput")
    out_d = nc.dram_tensor("out", [128, NBLK, K2], f32, kind="ExternalOutput")
    scr_d = nc.dram_tensor("scr", [NBLK * 128 * SPITCH], bf16)

    with tile.TileContext(nc) as tc:
        with (
            tc.tile_pool(name="resident", bufs=1) as res_pool,
            tc.tile_pool(name="yrow", bufs=2) as yrow_pool,
            tc.tile_pool(name="psum", bufs=8, space="PSUM") as psum_pool,
            tc.tile_pool(name="band", bufs=4) as band_pool,
            tc.tile_pool(name="rd", bufs=4) as rd_pool,
            tc.tile_pool(name="stage", bufs=3) as stage_pool,
        ):
            x_sb = res_pool.tile([C, NBLK * 128], bf16)
            y_sb = res_pool.tile([C, YW * YH], bf16)
            nc.sync.dma_start(x_sb[:], x_d[:])
            nc.sync.dma_start(y_sb[:], y_d[:])

            y_wm = y_sb[:].rearrange("c (w h) -> c w h", h=YH)

            blk = 0
            for bh in range(NBH):
                # stage w-major context row-band for this block row:
                # Yrow[c, w'*CH + h'] = y[c, w', 8*bh + h']
                yrow = yrow_pool.tile([C, YW * CH], bf16)
                nc.vector.tensor_copy(
                    yrow[:].rearrange("c (w h) -> c w h", h=CH),
                    y_wm[:, :, PH * bh:PH * bh + CH])
                for bw in range(NBW):
                    j = blk % FLUSH
                    if j == 0:
                        stage = stage_pool.tile([128, FLUSH * K2], f32)

                    lhsT = x_sb[:, blk * 128:(blk + 1) * 128]
                    rhs = yrow[:, PW * bw * CH:PW * bw * CH + NCTX]
                    ps = psum_pool.tile([128, NCTX], f32)
                    nc.tensor.matmul(ps[:], lhsT, rhs, start=True, stop=True)

                    # padded to 392 cols: keeps the partition step (392)
                    # != the 384-elem free span so the AP optimizer cannot
                    # merge the split partition dim into the free dim (that
                    # merge desyncs pairing with the unmergeable sheared
                    # dest AP and scrambles the write)
                    band = band_pool.tile([128, NCTX + 8], bf16)
                    if blk % 2 == 0:
                        nc.scalar.activation(
                            band[:, 0:NCTX], ps[:],
                            mybir.ActivationFunctionType.Copy,
                            scale=1.0 / C)
                    else:
                        nc.vector.tensor_scalar_mul(band[:, 0:NCTX], ps[:], 1.0 / C)

                    # sheared write: dest(hl, wl, n) =
                    #   blk*128*SPITCH + SHEAR_MAX + m*SPITCH + n - pos(m)
                    #   with m = hl*PW + wl, pos(m) = wl*CH + hl
                    # one 2D write per hl row-group: the single 3D form
                    # miscompiles in descriptor generation; the 2D custom-
                    # stride form is verified exact on hardware
                    w_list = []
                    for hl in range(PH):
                        dsth = _strided_view(
                            scr_d,
                            blk * 128 * SPITCH + SHEAR_MAX
                            + hl * (PW * SPITCH - 1),
                            [(SPITCH - CH, PW), (1, NCTX)])
                        w_list.append(nc.sync.dma_start(
                            dsth, band[hl * PW:(hl + 1) * PW, 0:NCTX]))

                    # uniform re-read of the sheared rows
                    rd = rd_pool.tile([128, K * CH], bf16)
                    rsrc = _strided_view(
                        scr_d, blk * 128 * SPITCH + SHEAR_MAX,
                        [(SPITCH, 128), (1, QW)])
                    r_ins = nc.sync.dma_start(rd[:, 0:QW], rsrc)
                    # custom APs defeat Tile's DRAM dep tracking: order the
                    # scratch RAW explicitly
                    from concourse.tile_rust import add_dep_helper
                    for w_ins in w_list:
                        add_dep_helper(r_ins.ins, w_ins.ins,
                                       reason="scratch sheared-band RAW")

                    # compact: out[p, di*9+dj] = rd[p, dj*CH + di], cast to f32
                    comp_src = rd[:].rearrange(
                        "p (a b) -> p a b", b=CH)[:, 0:K, 0:K]
                    dstage = stage[:, j * K2:(j + 1) * K2].rearrange(
                        "p (a b) -> p b a", b=K)
                    if blk % 2 == 0:
                        nc.vector.tensor_copy(dstage, comp_src)
                    else:
                        nc.scalar.activation(
                            dstage, comp_src,
                            mybir.ActivationFunctionType.Copy)

                    if j == FLUSH - 1:
                        nc.sync.dma_start(
                            out_d[:, blk - FLUSH + 1:blk + 1, :],
                            stage[:].rearrange("p (a b) -> p a b", b=K2))
                    blk += 1

    nc.compile()
    return nc


def _get_nc():
    global _nc_cache
    if _nc_cache is None:
        _nc_cache = _build_nc()
    return _nc_cache


def shard_inputs(x, y):
    import ml_dtypes
    xb = np.asarray(x).astype(ml_dtypes.bfloat16)
    yp = np.pad(np.asarray(y).astype(np.float32),
                ((0, 0), (0, 0), (PAD, PAD), (PAD, PAD))
                ).astype(ml_dtypes.bfloat16)
    in_maps = []
    for b in range(B):
        for hh in range(2):
            xs = xb[b, :, hh * HSH:(hh + 1) * HSH, :]     # [c, 96, 192]
            # pre-block: [c, bh, hl, bw, wl] -> [c, (bh bw), (hl wl)]
            xs = xs.reshape(C, NBH, PH, NBW, PW).transpose(0, 1, 3, 2, 4)
            xs = np.ascontiguousarray(xs.reshape(C, NBLK * 128))
            ys = yp[b, :, hh * HSH:hh * HSH + YH, :]      # [c, 104, 200]
            ys = np.ascontiguousarray(
                ys.transpose(0, 2, 1).reshape(C, YW * YH))  # w-major
            in_maps.append({"x": xs, "y": ys})
    return in_maps


def unshard_output(results):
    out = np.empty((B, K2, H, W), np.float32)
    for core, r in enumerate(results):
        o = np.asarray(r["out"])                 # [128, NBLK, 81]
        b, hh = divmod(core, 2)
        o = o.reshape(PH, PW, NBH, NBW, K2)      # [hl, wl, bh, bw, k]
        o = o.transpose(4, 2, 0, 3, 1).reshape(K2, HSH, W)
        out[b, :, hh * HSH:(hh + 1) * HSH, :] = o
    return out


def kernel(x, y, kernel_size, stride, _trace=False):
    assert int(kernel_size) == K and int(stride) == 1
    from concourse.bass_utils import run_bass_kernel_spmd
    nc = _get_nc()
    in_maps = shard_inputs(x, y)
    try:
        res = run_bass_kernel_spmd(nc, in_maps, list(range(NCORES)),
                                   trace=_trace)
    except Exception:
        if not _trace:
            raise
        res = run_bass_kernel_spmd(nc, in_maps, list(range(NCORES)))
    out = unshard_output(res.results)
    if _trace:
        return out, res
    return out



# revision 2
# speedup vs baseline: 11.1906x; 11.1906x over previous
"""FlowNet-style correlation layer (B=4, C=128, H=W=192, k=9, stride=1) on 8 trn2 cores.

Design (per core; cores = 4 batches x 2 H-halves, SPMD):
  - Host pre-blocks x into per-patch-contiguous layout [c, blk, 128] (bf16,
    pre-scaled by 1/C — exact in bf16) and pads y to [c, 104, 200] (bf16,
    h-major, zero-padded W and halo rows from the neighboring half).
  - Device: per 8x16 pixel patch (144 blocks), one PE matmul contracting
    channels: lhsT = x-patch [c, 128], rhs = strided 3D view of resident y
    [c, 16h, 24w] -> psum[128, 384] ("banded all-pairs": psum[m, n] =
    sum_c x[c,pix_m] * y[c,ctx_n]).
  - Evacuate psum -> staging SBUF bf16 (alternating ACT/DVE), flush G=8
    blocks per contiguous DMA to out DRAM [128, blk, 384].
  - Inputs are loaded in row-chunks so compute overlaps the load.
  - Host extracts the 81 useful offsets per pixel from the 384-wide band
    (numpy gather; n = (hl+i)*24 + (wl+j) for pixel (hl,wl), offset (i,j))
    and reassembles [B, 81, 192, 192] f32.

This keeps total DMA instructions ~O(50) (the v1 sheared-scratch design
dispatched ~1300 DMAs at ~600ns each on the sync engine = 786us serial).
"""

import numpy as np

B, C, H, W = 4, 128, 192, 192
K = 9                      # kernel_size
PAD = 4                    # displacement radius
NCORES = 8
HSH = H // 2               # 96 rows per core
YH, YW = HSH + 2 * PAD, W + 2 * PAD       # 104, 200
PH, PW = 8, 16             # patch shape (128 pixels)
CH, CW = PH + 2 * PAD, PW + 2 * PAD       # context 16 x 24
NCTX = CH * CW             # 384 band columns
NBH, NBW = HSH // PH, W // PW             # 12 x 12 = 144 blocks
NBLK = NBH * NBW
K2 = K * K                 # 81
FLUSH = 8                  # blocks per output flush

_nc_cache = None


def _build_nc():
    import concourse.bacc as bacc
    import concourse.mybir as mybir
    import concourse.tile as tile

    bf16 = mybir.dt.bfloat16
    f32 = mybir.dt.float32

    nc = bacc.Bacc("TRN2", target_bir_lowering=False, debug=False)
    x_d = nc.dram_tensor("x", [C, NBLK * 128], bf16, kind="ExternalInput")
    y_d = nc.dram_tensor("y", [C, YH * YW], bf16, kind="ExternalInput")
    out_d = nc.dram_tensor("out", [128, NBLK * NCTX], bf16,
                           kind="ExternalOutput")

    with tile.TileContext(nc) as tc:
        with (
            tc.tile_pool(name="resident", bufs=1) as res_pool,
            tc.tile_pool(name="psum", bufs=8, space="PSUM") as psum_pool,
            tc.tile_pool(name="stage", bufs=3) as stage_pool,
        ):
            x_sb = res_pool.tile([C, NBLK * 128], bf16)
            y_sb = res_pool.tile([C, YH * YW], bf16)

            # chunked loads (8 y rows / one x block-row per DMA) so the
            # first block rows can start while the tail still streams in
            YCH = 8 * YW                       # 1600 cols per y chunk
            XCH = NBW * 128                    # 1536 cols per x chunk
            for i in range(2):
                nc.sync.dma_start(y_sb[:, i * YCH:(i + 1) * YCH],
                                  y_d[:, i * YCH:(i + 1) * YCH])
            for bh in range(NBH):
                nc.sync.dma_start(x_sb[:, bh * XCH:(bh + 1) * XCH],
                                  x_d[:, bh * XCH:(bh + 1) * XCH])
                if bh + 2 < YH // 8:
                    i = bh + 2
                    nc.sync.dma_start(y_sb[:, i * YCH:(i + 1) * YCH],
                                      y_d[:, i * YCH:(i + 1) * YCH])

            y3 = y_sb[:].rearrange("c (h w) -> c h w", w=YW)

            blk = 0
            for bh in range(NBH):
                for bw in range(NBW):
                    j = blk % FLUSH
                    if j == 0:
                        stage = stage_pool.tile([128, FLUSH * NCTX], bf16)

                    lhsT = x_sb[:, blk * 128:(blk + 1) * 128]
                    rhs = y3[:, PH * bh:PH * bh + CH, PW * bw:PW * bw + CW]
                    ps = psum_pool.tile([128, NCTX], f32)
                    nc.tensor.matmul(ps[:], lhsT, rhs, start=True, stop=True)

                    dst = stage[:, j * NCTX:(j + 1) * NCTX]
                    if blk % 2 == 0:
                        nc.scalar.activation(
                            dst, ps[:], mybir.ActivationFunctionType.Copy)
                    else:
                        nc.vector.tensor_copy(dst, ps[:])

                    if j == FLUSH - 1:
                        nc.sync.dma_start(
                            out_d[:, (blk - FLUSH + 1) * NCTX:
                                  (blk + 1) * NCTX],
                            stage[:])
                    blk += 1

    nc.compile()
    return nc


def _get_nc():
    global _nc_cache
    if _nc_cache is None:
        _nc_cache = _build_nc()
    return _nc_cache


def shard_inputs(x, y):
    import ml_dtypes
    xs_all = (np.asarray(x) * np.float32(1.0 / C)).astype(ml_dtypes.bfloat16)
    yp = np.pad(np.asarray(y).astype(ml_dtypes.bfloat16),
                ((0, 0), (0, 0), (PAD, PAD), (PAD, PAD)))
    in_maps = []
    for b in range(B):
        for hh in range(2):
            xs = xs_all[b, :, hh * HSH:(hh + 1) * HSH, :]     # [c, 96, 192]
            # pre-block: [c, bh, hl, bw, wl] -> [c, (bh bw), (hl wl)]
            xs = xs.reshape(C, NBH, PH, NBW, PW).transpose(0, 1, 3, 2, 4)
            xs = np.ascontiguousarray(xs.reshape(C, NBLK * 128))
            ys = yp[b, :, hh * HSH:hh * HSH + YH, :]          # [c, 104, 200]
            ys = np.ascontiguousarray(ys.reshape(C, YH * YW))
            in_maps.append({"x": xs, "y": ys})
    return in_maps


# per-pixel band offset: pixel m = hl*PW + wl reads band cols
# pos(m) + i*CW + j  for offset k = i*K + j
_HL = np.arange(PH).repeat(PW)            # [128]
_WL = np.tile(np.arange(PW), PH)          # [128]
_POS = (_HL * CW + _WL).astype(np.int64)  # [128]
_OFF = (np.arange(K)[:, None] * CW + np.arange(K)).ravel()  # [81]
_IDX = np.broadcast_to((_POS[:, None] + _OFF[None, :])[:, None, :],
                       (128, NBLK, K2))


def unshard_output(results):
    out = np.empty((B, K2, H, W), np.float32)
    for core, r in enumerate(results):
        band = np.asarray(r["out"]).reshape(128, NBLK, NCTX)
        o = np.take_along_axis(band, _IDX, axis=2).astype(np.float32)
        b, hh = divmod(core, 2)
        o = o.reshape(PH, PW, NBH, NBW, K2)      # [hl, wl, bh, bw, k]
        o = o.transpose(4, 2, 0, 3, 1).reshape(K2, HSH, W)
        out[b, :, hh * HSH:(hh + 1) * HSH, :] = o
    return out


def kernel(x, y, kernel_size, stride, _trace=False):
    assert int(kernel_size) == K and int(stride) == 1
    from concourse.bass_utils import run_bass_kernel_spmd
    nc = _get_nc()
    in_maps = shard_inputs(x, y)
    try:
        res = run_bass_kernel_spmd(nc, in_maps, list(range(NCORES)),
                                   trace=_trace)
    except Exception:
        if not _trace:
            raise
        res = run_bass_kernel_spmd(nc, in_maps, list(range(NCORES)))
    out = unshard_output(res.results)
    if _trace:
        return out, res
    return out
